# revision 29
# baseline (speedup 1.0000x reference)
"""Trainium2 Bass kernel for the Clos-factorized MLP (nn_Clos_34282428956960).

The reference network
    h = x.reshape(b, c, 64, 64)                    # [b,c,n,r]
    h = einsum('bcnr,nrm->bcmr', h, w1) + bias1
    h = einsum('bcmr,rmn->bcnm', h, w2) + bias2
    h = einsum('bcnm,mro->bcor', h, w3) + bias3    # contracts BOTH n and m!
    y = h.reshape(b, c, -1)
collapses algebraically to a rank-256 linear map plus a constant:

    Y = X @ W1f @ W3f + c3tot
    W1f[d=(n,r), m] = w1[n,r,m] * w2s[r,m],  w2s = w2.sum(axis=2)
    W3f[m, o*64+r]  = w3[m,r,o]
    c3tot = (bias1 @ w2s + 64*bias2) @ W3f + tile(bias3, 64)

Device work per core (tokens sharded 8 ways): two fp16 GEMMs
    G = Xs @ W1s        Xs = (x/16) fp16 (pre-transposed on host)
    Yd = G  @ W3f       W1s = (16*W1f) fp16  ->  Yd = X @ W1f @ W3f exactly
with fp32 PSUM accumulation; Yd is written back as fp16 and the host
adds c3tot in fp32 (the constant dominates the output scale, so this
is also the accuracy-optimal place for it).

The 1/16 / x16 scaling keeps every fp16 operand comfortably inside the
normal range (W1f entries have std ~1.4e-4, below fp16's 6.1e-5 normal
floor without the scaling).

On-chip dataflow (per core, per 512-token chunk):
  MM1: G^T[m_p, t] += W1s[d_p, m].T @ Xs^T[d_p, t]   (32 d-tiles, kt-major
       so matmuls start as soon as each [128, 4kt, 512] x-slice DMA lands)
  G^T psum -> SBUF fp16 (DVE / ACT)
  MM2: Y[t_p, j]  += G^T[m_p, t].T @ W3f[m_p, j]     (2 m-tiles)
       psum -> SBUF fp16 converts alternate DVE/ACT; stores per
       (tt, j-half) so the final store is only 512 KB.
DMA order is just-in-time: w1 interleaved with x(ch0), then w3 (needed
only when MM2 starts), then x(ch1); y stores drain behind the loads.
"""

import numpy as np

TOK_TOTAL = 8192          # b*c = 2*4096 tokens
N_CORES = 8
TOK = TOK_TOTAL // N_CORES  # 1024 tokens per core
D = 4096                  # input features
M = 256                   # bottleneck
J = 4096                  # output features
ND = D // 128             # 32 d-tiles
CHUNK = 256               # tokens per chunk
NCH = TOK // CHUNK        # 2 chunks per core
TPC = CHUNK // 128        # 4 token-tiles per chunk
JT = 512                  # psum column tile
NJ = J // JT              # 8 j-tiles
JH = J // 2               # j-half for staged stores
XSCALE = 16.0             # fp16 dynamic-range scaling

_CACHE = {}


def _build_nc():
    import concourse.mybir as mybir
    import concourse.tile as tile
    from concourse import bacc

    F32 = mybir.dt.float32
    F16 = mybir.dt.float16

    nc = bacc.Bacc("TRN2", target_bir_lowering=False, debug=False,
                   num_devices=N_CORES)
    x = nc.dram_tensor("x", [NCH, 128, ND, CHUNK], F16, kind="ExternalInput")
    w1t = nc.dram_tensor("w1t", [128, 2, ND, 128], F16, kind="ExternalInput")
    w3t = nc.dram_tensor("w3t", [128, 2, J], F16, kind="ExternalInput")
    y = nc.dram_tensor("y", [TOK, J], F16, kind="ExternalOutput")

    with tile.TileContext(nc) as tc:
        with (
            tc.tile_pool(name="const", bufs=1) as const_pool,
            tc.tile_pool(name="xin", bufs=1) as xin_pool,
            tc.tile_pool(name="gt", bufs=2) as gt_pool,
            tc.tile_pool(name="yout", bufs=6) as yout_pool,
            tc.tile_pool(name="ytail", bufs=8) as ytail_pool,
            tc.tile_pool(name="g_psum", bufs=2, space="PSUM") as g_psum,
            tc.tile_pool(name="y_psum", bufs=6, space="PSUM") as y_psum,
        ):
            # ---- all load DMAs up front (SP sequencer never blocks them) ----
            w1_sb = const_pool.tile([128, 2, ND, 128], F16)
            w3_sb = const_pool.tile([128, 2, J], F16)
            xts = [xin_pool.tile([128, ND, CHUNK], F16, name=f"xt{c}",
                                 tag=f"xt{c}") for c in range(NCH)]
            # fine-grained first slices so the first matmul starts ~3us in,
            # coarser afterwards; w3 interleaved with x(ch1) so MM2(ch0) can
            # start right after MM1(ch0)
            xsl = [(0, 4), (4, 8), (8, 16), (16, 24), (24, 32)]
            for k0, k1 in xsl:
                nc.sync.dma_start(w1_sb[:, 0, k0:k1, :], w1t[:, 0, k0:k1, :])
                nc.sync.dma_start(xts[0][:, k0:k1, :], x[0, :, k0:k1, :])
            nc.sync.dma_start(w1_sb[:, 1, :, :], w1t[:, 1, :, :])
            nc.sync.dma_start(w3_sb[:, :, 0:JH], w3t[:, :, 0:JH])
            nc.sync.dma_start(xts[1][:, 0:16, :], x[1, :, 0:16, :])
            nc.sync.dma_start(w3_sb[:, :, JH:J], w3t[:, :, JH:J])
            nc.sync.dma_start(xts[1][:, 16:32, :], x[1, :, 16:32, :])
            for ch in range(2, NCH):
                for s in range(ND // 16):
                    k0 = 16 * s
                    nc.sync.dma_start(xts[ch][:, k0:k0 + 16, :],
                                      x[ch, :, k0:k0 + 16, :])

            # ---- compute emission helpers ----
            gts = {}
            state = {"cp": 0}

            def mm1(ch):
                """MM1 for chunk ch + gt convert (DVE then ACT)."""
                xt = xts[ch]
                gps = [g_psum.tile([128, CHUNK], F32, name=f"gp{mt}",
                                   tag="gp") for mt in range(2)]
                for kt in range(ND):
                    for mt in range(2):
                        nc.tensor.matmul(
                            gps[mt][:],
                            w1_sb[:, mt, kt, :],
                            xt[:, kt, :],
                            start=(kt == 0), stop=(kt == ND - 1))
                gt = gt_pool.tile([128, 2, CHUNK], F16)
                nc.vector.tensor_copy(gt[:, 0, :], gps[0][:])
                nc.scalar.copy(gt[:, 1, :], gps[1][:])
                gts[ch] = gt

            def mm2_unit(ch, tt, j0, spj):
                """One yo tile: matmuls + converts (single engine) + store."""
                cp = state["cp"]
                gt = gts[ch]
                row0 = ch * CHUNK + tt * 128
                pool = ytail_pool if spj == JT else yout_pool
                yo = pool.tile([128, spj], F16, name=f"yo{spj}",
                               tag=f"yo{spj}")
                eng = nc.vector.tensor_copy if cp % 2 == 0 else nc.scalar.copy
                for jq in range(spj // JT):
                    jt = (j0 + jq * JT) // JT
                    yp = y_psum.tile([128, JT], F32)
                    for mt in range(2):
                        nc.tensor.matmul(
                            yp[:],
                            gt[:, mt, tt * 128:(tt + 1) * 128],
                            w3_sb[:, mt, jt * JT:(jt + 1) * JT],
                            start=(mt == 0), stop=(mt == 1))
                    eng(yo[:, jq * JT:(jq + 1) * JT], yp[:])
                st = nc.sync if (spj == JT and cp % 2 == 1) else nc.gpsimd
                st.dma_start(y[row0:row0 + 128, j0:j0 + spj], yo[:])
                state["cp"] = cp + 1

            def mm2_units(ch):
                units = []
                for tt in range(TPC):
                    last = (ch == NCH - 1 and tt == TPC - 1)
                    spj = JT if last else JH
                    for jh in range(J // spj):
                        units.append((ch, tt, jh * spj, spj))
                return units

            # ---- staggered schedule: each chunk's last MM2 unit is emitted
            # after the NEXT chunk's MM1, hiding the gt-convert latency at
            # every chunk transition and keeping stores flowing during MM1
            mm1(0)
            for ch in range(NCH):
                units = mm2_units(ch)
                for u in units[:-1]:
                    mm2_unit(*u)
                if ch + 1 < NCH:
                    mm1(ch + 1)
                mm2_unit(*units[-1])
    nc.compile()
    return nc


def _fold_weights(w1, w2, w3, bias1, bias2, bias3):
    """Collapse the 3-stage Clos into W1f [4096,256], W3f [256,4096], c3tot."""
    w1 = np.asarray(w1, np.float64)
    w2 = np.asarray(w2, np.float64)
    w3 = np.asarray(w3, np.float64)
    b1 = np.asarray(bias1, np.float64)
    b2 = np.asarray(bias2, np.float64)
    b3 = np.asarray(bias3, np.float64)

    w2s = w2.sum(axis=2)                                   # [64(r), 256(m)]
    W1f = (w1 * w2s[None, :, :]).reshape(D, M)             # [(n,r), m]
    c2 = b1 @ w2s + w2.shape[2] * b2                       # [256]
    W3f = np.transpose(w3, (0, 2, 1)).reshape(M, J)        # [m, (o,r)]
    c3tot = c2 @ W3f + np.tile(b3, J // b3.shape[0])       # [4096]
    return W1f, W3f, c3tot


def _prepare(x, w1, w2, w3, bias1, bias2, bias3):
    """Host-side prep: fold weights, scale, transpose, downcast to fp16.

    Returns (in_maps, c3tot) where in_maps[i] feeds core i.
    """
    W1f, W3f, c3tot = _fold_weights(w1, w2, w3, bias1, bias2, bias3)
    w1t = np.ascontiguousarray(
        (W1f * XSCALE).reshape(ND, 128, 2, 128).transpose(1, 2, 0, 3)
    ).astype(np.float16)
    w3t = np.ascontiguousarray(
        W3f.reshape(2, 128, J).transpose(1, 0, 2)).astype(np.float16)

    xs = (np.asarray(x, np.float32).reshape(TOK_TOTAL, D)
          * np.float32(1.0 / XSCALE)).astype(np.float16)
    in_maps = []
    for i in range(N_CORES):
        xc = xs[i * TOK:(i + 1) * TOK]                    # [1024, 4096]
        # [NCH, 128(dp), ND(kt), CHUNK(t)]: xT per chunk, kt-major partitions
        xr = (xc.reshape(NCH, CHUNK, ND, 128)
              .transpose(0, 3, 2, 1))
        in_maps.append({"x": np.ascontiguousarray(xr),
                        "w1t": w1t, "w3t": w3t})
    return in_maps, c3tot.astype(np.float32)


def kernel(x, w1, w2, w3, bias1, bias2, bias3):
    from concourse.bass_utils import run_bass_kernel_spmd

    in_maps, c3tot = _prepare(x, w1, w2, w3, bias1, bias2, bias3)

    if "nc" not in _CACHE:
        _CACHE["nc"] = _build_nc()
    nc = _CACHE["nc"]

    res = run_bass_kernel_spmd(nc, in_maps, core_ids=list(range(N_CORES)))
    y = np.concatenate([res.results[i]["y"] for i in range(N_CORES)], axis=0)
    y = y.astype(np.float32)
    y += c3tot[None, :]
    return y.reshape(x.shape[0], x.shape[1], J)


# revision 30
# speedup vs baseline: 1.7163x; 1.7163x over previous
"""Trainium2 Bass kernel for the Clos-factorized MLP (nn_Clos_34282428956960).

The reference network
    h = x.reshape(b, c, 64, 64)                    # [b,c,n,r]
    h = einsum('bcnr,nrm->bcmr', h, w1) + bias1
    h = einsum('bcmr,rmn->bcnm', h, w2) + bias2
    h = einsum('bcnm,mro->bcor', h, w3) + bias3    # contracts BOTH n and m!
    y = h.reshape(b, c, -1)
collapses algebraically to a rank-256 linear map plus a constant:

    Y = X @ W1f @ W3f + c3tot
    W1f[d=(n,r), m] = w1[n,r,m] * w2s[r,m],  w2s = w2.sum(axis=2)
    W3f[m, o*64+r]  = w3[m,r,o]
    c3tot = (bias1 @ w2s + 64*bias2) @ W3f + tile(bias3, 64)

Device work per core (tokens sharded 8 ways): two fp16 GEMMs
    G = Xs @ W1s        Xs = (x/16) fp16 (pre-transposed on host)
    Yd = G  @ W3f       W1s = (16*W1f) fp16  ->  Yd = X @ W1f @ W3f exactly
with fp32 PSUM accumulation; Yd is written back as fp16 and the host
adds c3tot in fp32 (the constant dominates the output scale, so this
is also the accuracy-optimal place for it).

The 1/16 / x16 scaling keeps every fp16 operand comfortably inside the
normal range (W1f entries have std ~1.4e-4, below fp16's 6.1e-5 normal
floor without the scaling).

On-chip dataflow (per core, per 512-token chunk):
  MM1: G^T[m_p, t] += W1s[d_p, m].T @ Xs^T[d_p, t]   (32 d-tiles, kt-major
       so matmuls start as soon as each [128, 4kt, 512] x-slice DMA lands)
  G^T psum -> SBUF fp16 (DVE / ACT)
  MM2: Y[t_p, j]  += G^T[m_p, t].T @ W3f[m_p, j]     (2 m-tiles)
       psum -> SBUF fp16 converts alternate DVE/ACT; stores per
       (tt, j-half) so the final store is only 512 KB.
DMA order is just-in-time: w1 interleaved with x(ch0), then w3 (needed
only when MM2 starts), then x(ch1); y stores drain behind the loads.
"""

import numpy as np

TOK_TOTAL = 8192          # b*c = 2*4096 tokens
N_CORES = 8
TOK = TOK_TOTAL // N_CORES  # 1024 tokens per core
D = 4096                  # input features
M = 256                   # bottleneck
J = 4096                  # output features
ND = D // 128             # 32 d-tiles
CHUNK = 256               # tokens per chunk
NCH = TOK // CHUNK        # 2 chunks per core
TPC = CHUNK // 128        # 4 token-tiles per chunk
JT = 512                  # psum column tile
NJ = J // JT              # 8 j-tiles
JH = J // 2               # j-half for staged stores
XSCALE = 2.0              # x scale: x/2 in fp8 (sigma 0.5)
W1SCALE = 1024.0          # W1f*1024 in fp8; psum_g = 512*G
W3SCALE = 64.0            # W3f*64 in fp8; psum_y = (16G)@(64W3f) = 1024*y_lin
YSCALE = 1024.0           # host divides y by this

_CACHE = {}


def _build_nc():
    import concourse.mybir as mybir
    import concourse.tile as tile
    from concourse import bacc

    F32 = mybir.dt.float32
    F16 = mybir.dt.float16
    F8 = mybir.dt.float8e4
    DR = mybir.MatmulPerfMode.DoubleRow

    nc = bacc.Bacc("TRN2", target_bir_lowering=False, debug=False,
                   num_devices=N_CORES)
    x = nc.dram_tensor("x", [NCH, 128, ND, CHUNK], F8, kind="ExternalInput")
    w1t = nc.dram_tensor("w1t", [128, ND, M], F8, kind="ExternalInput")
    w3t = nc.dram_tensor("w3t", [128, 2, J], F8, kind="ExternalInput")
    y = nc.dram_tensor("y", [TOK, J], F8, kind="ExternalOutput")

    with tile.TileContext(nc) as tc:
        with (
            tc.tile_pool(name="const", bufs=1) as const_pool,
            tc.tile_pool(name="xin", bufs=1) as xin_pool,
            tc.tile_pool(name="gt", bufs=2) as gt_pool,
            tc.tile_pool(name="yout", bufs=6) as yout_pool,
            tc.tile_pool(name="ytail", bufs=8) as ytail_pool,
            tc.tile_pool(name="g_psum", bufs=2, space="PSUM") as g_psum,
            tc.tile_pool(name="y_psum", bufs=6, space="PSUM") as y_psum,
        ):
            # ---- all load DMAs up front (SP sequencer never blocks them) ----
            w1_sb = const_pool.tile([128, ND, M], F8)
            w3_sb = const_pool.tile([128, 2, J], F8)
            xts = [xin_pool.tile([128, ND, CHUNK], F8, name=f"xt{c}",
                                 tag=f"xt{c}") for c in range(NCH)]
            # fine-grained first slices so the first matmul starts ~3us in,
            # coarser afterwards; w3 interleaved with x(ch1) so MM2(ch0) can
            # start right after MM1(ch0)
            xsl = [(0, 4), (4, 8), (8, 16), (16, 24), (24, 32)]
            for k0, k1 in xsl:
                nc.sync.dma_start(w1_sb[:, k0:k1, :], w1t[:, k0:k1, :])
                nc.sync.dma_start(xts[0][:, k0:k1, :], x[0, :, k0:k1, :])
            nc.sync.dma_start(w3_sb[:, :, 0:JH], w3t[:, :, 0:JH])
            nc.sync.dma_start(xts[1][:, 0:16, :], x[1, :, 0:16, :])
            nc.sync.dma_start(w3_sb[:, :, JH:J], w3t[:, :, JH:J])
            nc.sync.dma_start(xts[1][:, 16:32, :], x[1, :, 16:32, :])
            for ch in range(2, NCH):
                for s in range(ND // 16):
                    k0 = 16 * s
                    nc.sync.dma_start(xts[ch][:, k0:k0 + 16, :],
                                      x[ch, :, k0:k0 + 16, :])

            # ---- compute emission helpers ----
            gts = {}
            state = {"cp": 0}

            def mm1(ch):
                """MM1 for chunk ch (fp8 DoubleRow, kt pairs) + gt convert."""
                xt = xts[ch]
                gps = [g_psum.tile([128, CHUNK], F32, name=f"gp{mt}",
                                   tag="gp") for mt in range(2)]
                for kp in range(ND // 2):
                    kt = 2 * kp
                    for mt in range(2):
                        nc.tensor.matmul(
                            gps[mt][:],
                            w1_sb[:, kt:kt + 2, mt * 128:(mt + 1) * 128],
                            xt[:, kt:kt + 2, :],
                            start=(kp == 0), stop=(kp == ND // 2 - 1),
                            perf_mode=DR)
                # gt = psum/32 = 16*G in fp8 (psum is 512*G)
                gt = gt_pool.tile([128, 2, CHUNK], F8)
                nc.vector.tensor_scalar_mul(gt[:, 0, :], gps[0][:], 1.0 / 32)
                nc.scalar.mul(gt[:, 1, :], gps[1][:], 1.0 / 32)
                gts[ch] = gt

            def mm2_unit(ch, tt, j0, spj):
                """One yo tile: matmuls + converts (single engine) + store."""
                cp = state["cp"]
                gt = gts[ch]
                row0 = ch * CHUNK + tt * 128
                pool = ytail_pool if spj == JT else yout_pool
                yo = pool.tile([128, spj], F8, name=f"yo{spj}",
                               tag=f"yo{spj}")
                eng = nc.vector.tensor_copy if cp % 2 == 0 else nc.scalar.copy
                for jq in range(spj // JT):
                    jt = (j0 + jq * JT) // JT
                    yp = y_psum.tile([128, JT], F32)
                    nc.tensor.matmul(
                        yp[:],
                        gt[:, :, tt * 128:(tt + 1) * 128],
                        w3_sb[:, :, jt * JT:(jt + 1) * JT],
                        start=True, stop=True, perf_mode=DR)
                    eng(yo[:, jq * JT:(jq + 1) * JT], yp[:])
                st = nc.sync if (spj == JT and cp % 2 == 1) else nc.gpsimd
                st.dma_start(y[row0:row0 + 128, j0:j0 + spj], yo[:])
                state["cp"] = cp + 1

            def mm2_units(ch):
                units = []
                for tt in range(TPC):
                    last = (ch == NCH - 1 and tt == TPC - 1)
                    spj = JT if last else JH
                    for jh in range(J // spj):
                        units.append((ch, tt, jh * spj, spj))
                return units

            # ---- staggered schedule: each chunk's last MM2 unit is emitted
            # after the NEXT chunk's MM1, hiding the gt-convert latency at
            # every chunk transition and keeping stores flowing during MM1
            mm1(0)
            for ch in range(NCH):
                units = mm2_units(ch)
                for u in units[:-1]:
                    mm2_unit(*u)
                if ch + 1 < NCH:
                    mm1(ch + 1)
                mm2_unit(*units[-1])
    nc.compile()
    return nc


def _fold_weights(w1, w2, w3, bias1, bias2, bias3):
    """Collapse the 3-stage Clos into W1f [4096,256], W3f [256,4096], c3tot."""
    w1 = np.asarray(w1, np.float64)
    w2 = np.asarray(w2, np.float64)
    w3 = np.asarray(w3, np.float64)
    b1 = np.asarray(bias1, np.float64)
    b2 = np.asarray(bias2, np.float64)
    b3 = np.asarray(bias3, np.float64)

    w2s = w2.sum(axis=2)                                   # [64(r), 256(m)]
    W1f = (w1 * w2s[None, :, :]).reshape(D, M)             # [(n,r), m]
    c2 = b1 @ w2s + w2.shape[2] * b2                       # [256]
    W3f = np.transpose(w3, (0, 2, 1)).reshape(M, J)        # [m, (o,r)]
    c3tot = c2 @ W3f + np.tile(b3, J // b3.shape[0])       # [4096]
    return W1f, W3f, c3tot


def _prepare(x, w1, w2, w3, bias1, bias2, bias3):
    """Host-side prep: fold weights, scale, transpose, downcast to fp16.

    Returns (in_maps, c3tot) where in_maps[i] feeds core i.
    """
    import ml_dtypes
    F8NP = ml_dtypes.float8_e4m3
    W1f, W3f, c3tot = _fold_weights(w1, w2, w3, bias1, bias2, bias3)
    w1t = np.ascontiguousarray(
        (W1f * W1SCALE).reshape(ND, 128, M).transpose(1, 0, 2)
    ).astype(np.float32).astype(F8NP)
    w3t = np.ascontiguousarray(
        (W3f * W3SCALE).reshape(2, 128, J).transpose(1, 0, 2)
    ).astype(np.float32).astype(F8NP)

    xs = (np.asarray(x, np.float32).reshape(TOK_TOTAL, D)
          * np.float32(1.0 / XSCALE)).astype(F8NP)
    in_maps = []
    for i in range(N_CORES):
        xc = xs[i * TOK:(i + 1) * TOK]                    # [1024, 4096]
        # [NCH, 128(dp), ND(kt), CHUNK(t)]: xT per chunk, kt-major partitions
        xr = (xc.reshape(NCH, CHUNK, ND, 128)
              .transpose(0, 3, 2, 1))
        in_maps.append({"x": np.ascontiguousarray(xr),
                        "w1t": w1t, "w3t": w3t})
    return in_maps, c3tot.astype(np.float32)


def kernel(x, w1, w2, w3, bias1, bias2, bias3):
    from concourse.bass_utils import run_bass_kernel_spmd

    in_maps, c3tot = _prepare(x, w1, w2, w3, bias1, bias2, bias3)

    if "nc" not in _CACHE:
        _CACHE["nc"] = _build_nc()
    nc = _CACHE["nc"]

    res = run_bass_kernel_spmd(nc, in_maps, core_ids=list(range(N_CORES)))
    y = np.concatenate([res.results[i]["y"] for i in range(N_CORES)], axis=0)
    y = y.astype(np.float32)
    y *= np.float32(1.0 / YSCALE)
    y += c3tot[None, :]
    return y.reshape(x.shape[0], x.shape[1], J)
